# revision 21
# baseline (speedup 1.0000x reference)
"""Bilinear grid_sample (zeros padding, align_corners=False) Bass kernel for TRN2.

Per-core problem: x [64, H*W] f32 (NCHW flattened), gxy [128, 2*NT] f32
(host-transposed grid planes; cols 0:NT = gx, NT:2NT = gy, where plane[p, t]
= grid[t*128 + p]), out [64, H*W] f32.

Strategy (v2 — batched SWDGE dma_gather):
  Logical pair-table TBL[rho] = {upper: x_nhwc[rho-W-1], lower: x_nhwc[rho-1]}
  (zeros outside [0, HW)); per pixel r = (yc'+1)*W + xc' + 1 and the 4 bilinear
  taps are the upper/lower halves of TBL[r] and TBL[r+1].  To fit int16 gather
  indices, TBL is split into parity tables of 32768 rows each:
     O[k] = TBL[2k+1]   E[k] = TBL[2k+2]
  and the per-axis edge case (floor == size-1, where the +1 tap has zero
  weight) is remapped one step down so r <= 65535.  Each pixel gathers one O
  row (kO = floor(r/2)) and one E row (kE = max(0, floor((r-1)/2))); which
  table carries the {r} vs {r+1} slots depends on parity(r), folded into the
  per-pixel weights.  Tables are stored permuted (k' = (k%128)*NB + k//128) so
  the build writes 2 KB contiguous per partition.  Gathers run as large
  InstDMAGatherAnt batches (idx int16, replicated across the 8 Q7 16-partition
  stripes; idx j of a call sits at [16c + j%16, j//16] for all c).  Weighted
  sum on DVE, TensorE transpose back to [ch, px], ScalarE PSUM-evict cast
  fp16->f32, DMA out.
"""

from contextlib import ExitStack

import numpy as np

import concourse.bass as bass
import concourse.tile as tile
from concourse import mybir
from concourse.library_config import mlp
from concourse.masks import make_identity

F32 = mybir.dt.float32
F16 = mybir.dt.float16
I32 = mybir.dt.int32
I16 = mybir.dt.int16
MUL = mybir.AluOpType.mult
ADD = mybir.AluOpType.add
SUB = mybir.AluOpType.subtract
MAX = mybir.AluOpType.max
MIN = mybir.AluOpType.min
GE = mybir.AluOpType.is_ge
LE = mybir.AluOpType.is_le
GT = mybir.AluOpType.is_gt


def gs_body(ctx: ExitStack, tc: tile.TileContext, out_ap, x_ap, gxy_ap, *,
            H=256, W=256, GK=32, K=16, single_packet=False, n_queues=1,
            debug_taps=None):
    nc = tc.nc
    C = 64
    HW = H * W
    NT = HW // 128            # 128-px tiles (weight-plane columns)
    NROWS = HW // 2           # rows per parity table
    NB = NROWS // 128         # 128-row blocks per table
    SBK = 8                   # blocks per build superblock
    NSB = NB // SBK           # build superblocks
    SPX = HW // NSB           # x pixels per superblock

    nc.gpsimd.load_library(mlp)

    tabO = nc.dram_tensor("tabO", [NROWS, 2 * C], F16, kind="Internal").ap()
    tabE = nc.dram_tensor("tabE", [NROWS, 2 * C], F16, kind="Internal").ap()

    persist = ctx.enter_context(tc.tile_pool(name="persist", bufs=1))
    loadp = ctx.enter_context(tc.tile_pool(name="loadp", bufs=3))
    psumb = ctx.enter_context(tc.tile_pool(name="psumb", bufs=2, space="PSUM"))
    psumi = ctx.enter_context(tc.tile_pool(name="psumi", bufs=2, space="PSUM"))
    fusedp = ctx.enter_context(tc.tile_pool(name="fusedp", bufs=4))
    gath = ctx.enter_context(tc.tile_pool(name="gath", bufs=2))
    accp = ctx.enter_context(tc.tile_pool(name="accp", bufs=2))
    psumo = ctx.enter_context(tc.tile_pool(name="psumo", bufs=2, space="PSUM"))
    outp = ctx.enter_context(tc.tile_pool(name="outp", bufs=3))

    ident32 = persist.tile([128, 128], F32)
    make_identity(nc, ident32[:])
    ident16 = persist.tile([128, 128], F16)
    make_identity(nc, ident16[:])
    # S_q[p, m] = 1 iff p == 16q + m%16 (selectors for idx replication
    # matmuls; PE needs lhsT/rhs base partition 0 so contract over all 128)
    smat = persist.tile([128, 8 * 128], F32)
    for q in range(8):
        for rep in range(8):
            nc.vector.tensor_copy(
                smat[:, 128 * q + 16 * rep:128 * q + 16 * (rep + 1)],
                ident32[:, 16 * q:16 * (q + 1)])

    # ---------------- prologue: grid -> weights + gather indices ----------
    g_sb = persist.tile([128, 2 * NT], F32)
    nc.sync.dma_start(g_sb[:], gxy_ap[:])

    def ptile(dt, name):
        return persist.tile([128, NT], dt, name=name, tag=name)

    # shared scratch planes: every prologue op runs serially on DVE, so
    # aliasing temps through fixed tags costs no parallelism, only SBUF.
    scr_i = ptile(I32, "scr_i")
    ta = ptile(F32, "ta")
    tb = ptile(F32, "tb")
    tc_ = ptile(F32, "tc")

    def ffloor(dst_f32, v_ap):
        """dst = floor(v); v >= -1 smallish ints, exact in f32."""
        nc.vector.tensor_copy(scr_i[:], v_ap)
        nc.vector.tensor_copy(dst_f32, scr_i[:])
        nc.vector.tensor_tensor(ta[:], dst_f32, v_ap, op=GT)
        nc.vector.tensor_tensor(dst_f32, dst_f32, ta[:], op=SUB)

    def axis_prep(gsl, size, ax):
        """Return (c0, c1, clamped+edge-shifted floor in [-1, size-2])."""
        v = ptile(F32, f"v_{ax}")       # dies at end of this call
        nc.vector.tensor_scalar(v[:], gsl, size / 2.0, (size - 1) / 2.0, MUL, ADD)
        vf = ptile(F32, f"vf_{ax}")
        ffloor(vf[:], v[:])
        t = ptile(F32, f"t_{ax}")
        nc.vector.tensor_tensor(t[:], v[:], vf[:], op=SUB)
        nc.vector.tensor_scalar(tb[:], vf[:], 0.0, None, GE)
        nc.vector.tensor_scalar(tc_[:], vf[:], size - 1.0, None, LE)
        nc.vector.tensor_tensor(tb[:], tb[:], tc_[:], op=MUL)     # valid0
        w0 = v                                                     # reuse
        nc.vector.tensor_scalar(w0[:], t[:], -1.0, 1.0, MUL, ADD)
        nc.vector.tensor_tensor(w0[:], w0[:], tb[:], op=MUL)      # (1-t)*v0
        nc.vector.tensor_scalar(tb[:], vf[:], -1.0, None, GE)
        nc.vector.tensor_scalar(tc_[:], vf[:], size - 2.0, None, LE)
        nc.vector.tensor_tensor(tb[:], tb[:], tc_[:], op=MUL)     # valid1
        nc.vector.tensor_tensor(t[:], t[:], tb[:], op=MUL)        # t*v1
        nc.vector.tensor_scalar(vf[:], vf[:], -1.0, size - 1.0, MAX, MIN)
        # edge shift: s = (vf == size-1) -> use the size-2 tap pair instead
        nc.vector.tensor_scalar(tb[:], vf[:], size - 1.0, None, GE)   # s
        nc.vector.tensor_scalar(tc_[:], tb[:], -1.0, 1.0, MUL, ADD)   # 1-s
        c0 = ptile(F32, f"c0_{ax}")
        nc.vector.tensor_tensor(c0[:], w0[:], tc_[:], op=MUL)
        c1 = ptile(F32, f"c1_{ax}")
        nc.vector.tensor_tensor(c1[:], t[:], tc_[:], op=MUL)
        nc.vector.tensor_tensor(ta[:], w0[:], tb[:], op=MUL)      # w0*s
        nc.vector.tensor_tensor(c1[:], c1[:], ta[:], op=ADD)
        nc.vector.tensor_tensor(vf[:], vf[:], tb[:], op=SUB)
        return c0, c1, vf

    cx0, cx1, xc = axis_prep(g_sb[:, 0:NT], float(W), "x")
    cy0, cy1, yc = axis_prep(g_sb[:, NT:2 * NT], float(H), "y")

    # r = yc*W + xc + W + 1  in [0, 65535]
    rf = ptile(F32, "rf")
    nc.vector.tensor_scalar(rf[:], yc[:], float(W), W + 1.0, MUL, ADD)
    nc.vector.tensor_tensor(rf[:], rf[:], xc[:], op=ADD)
    # h = floor(r/2), par = r - 2h
    hf = ptile(F32, "hf")
    nc.vector.tensor_scalar(tb[:], rf[:], 0.5, None, MUL)
    ffloor(hf[:], tb[:])
    par = ptile(F32, "par")
    nc.vector.tensor_scalar(par[:], hf[:], -2.0, None, MUL)
    nc.vector.tensor_tensor(par[:], rf[:], par[:], op=ADD)
    # kO = h ; kE = max(0, h + par - 1)
    ke = rf                                                       # reuse
    nc.vector.tensor_tensor(ke[:], hf[:], par[:], op=ADD)
    nc.vector.tensor_scalar(ke[:], ke[:], -1.0, 0.0, ADD, MAX)

    def permute_rows(kf, name):
        """k' = (k % 128)*NB + k//128 (table stored partition-major)."""
        kp = ptile(F32, f"kp_{name}")
        nc.vector.tensor_scalar(tb[:], kf[:], 1.0 / 128.0, None, MUL)
        ffloor(kp[:], tb[:])                                      # kp = k//128
        nc.vector.tensor_scalar(tb[:], kp[:], -128.0, None, MUL)
        nc.vector.tensor_tensor(tb[:], kf[:], tb[:], op=ADD)      # k%128
        nc.vector.tensor_scalar(tb[:], tb[:], float(NB), None, MUL)
        nc.vector.tensor_tensor(kp[:], tb[:], kp[:], op=ADD)
        return kp

    kpO = permute_rows(hf, "O")
    kpE = permute_rows(ke, "E")

    # slot weights + parity blend (fp16, interleaved [p, (t s)] with s=up,lo)
    s0 = ptile(F32, "s0")
    s1 = ptile(F32, "s1")
    s2 = ptile(F32, "s2")
    s3 = ptile(F32, "s3")
    nc.vector.tensor_tensor(s0[:], cy0[:], cx0[:], op=MUL)
    nc.vector.tensor_tensor(s1[:], cy1[:], cx0[:], op=MUL)
    nc.vector.tensor_tensor(s2[:], cy0[:], cx1[:], op=MUL)
    nc.vector.tensor_tensor(s3[:], cy1[:], cx1[:], op=MUL)
    omp = ptile(F32, "omp")
    nc.vector.tensor_scalar(omp[:], par[:], -1.0, 1.0, MUL, ADD)
    wE = persist.tile([128, NT * 2], F16)
    wO = persist.tile([128, NT * 2], F16)
    wEv = wE[:].rearrange("p (t s) -> p t s", s=2)
    wOv = wO[:].rearrange("p (t s) -> p t s", s=2)
    # E carries {S0,S1} when r even, {S2,S3} when r odd; O the reverse.
    for dstv, a_even, a_odd in (
        (wEv[:, :, 0:1], s0, s2),
        (wEv[:, :, 1:2], s1, s3),
        (wOv[:, :, 0:1], s2, s0),
        (wOv[:, :, 1:2], s3, s1),
    ):
        nc.vector.tensor_tensor(ta[:], a_even[:], omp[:], op=MUL)
        nc.vector.tensor_tensor(tb[:], a_odd[:], par[:], op=MUL)
        nc.vector.tensor_tensor(ta[:], ta[:], tb[:], op=ADD)
        nc.vector.tensor_copy(dstv.squeeze(2), ta[:])

    if debug_taps is not None:
        debug_taps["wE"] = wE[:]
        debug_taps["wO"] = wO[:]

    # ---------------- idx planes: int16, stripe-replicated ----------------
    # idx for call-slot j lives at [16c + j%16, j//16] for every stripe c.
    # j = t*128 + p  ->  partition (j%16)=p%16 rep 8x, column 8t + p//16.
    idxO = persist.tile([128, 8 * NT], I16)
    idxE = persist.tile([128, 8 * NT], I16)
    for kp, idx in ((kpO, idxO), (kpE, idxE)):
        idxv = idx[:].rearrange("p (t q) -> p t q", q=8)
        for q in range(8):
            pm = psumi.tile([128, NT], F32, tag="pmidx")
            nc.tensor.matmul(out=pm[:], lhsT=smat[:, 128 * q:128 * (q + 1)],
                             rhs=kp[:], start=True, stop=True)
            nc.vector.tensor_copy(idxv[:, :, q:q + 1].squeeze(2), pm[:])

    # ---------------- build the parity tables -----------------------------
    tabOv = tabO.rearrange("(p B) e -> p B e", p=128)
    tabEv = tabE.rearrange("(p B) e -> p B e", p=128)
    prev_ps = [None, None]
    for b in range(NSB):
        xs = loadp.tile([C, SPX], F32)
        nc.sync.dma_start(xs[:], x_ap[:, SPX * b:SPX * (b + 1)])
        xsv = xs[:].rearrange("c (px two) -> c px two", two=2)
        for parity, tabv in ((0, tabOv), (1, tabEv)):
            ps = psumb.tile([128, SBK * C], F32, tag=f"ps{parity}")
            for j in range(SBK):
                nc.tensor.transpose(
                    ps[:, C * j:C * (j + 1)],
                    xsv[:, 128 * j:128 * (j + 1), parity:parity + 1].squeeze(2),
                    ident32[0:C, 0:C])
            psv = ps[:].rearrange("p (j c) -> p j c", c=C)
            fused = fusedp.tile([128, SBK, 2 * C], F16, tag=f"fu{parity}")
            # lowers
            nc.scalar.activation(fused[:, :, C:2 * C], psv,
                                 mybir.ActivationFunctionType.Copy)
            # uppers j>=1 from this superblock, j=0 from the previous one
            nc.vector.tensor_copy(fused[:, 1:SBK, 0:C], psv[:, 0:SBK - 1, :])
            if b == 0:
                nc.gpsimd.memset(fused[:, 0:1, 0:C], 0.0)
            else:
                nc.vector.tensor_copy(fused[:, 0:1, 0:C],
                                      prev_ps[parity][:, SBK - 1:SBK, :])
            prev_ps[parity] = psv
            eng = nc.sync if parity == 0 else nc.scalar
            eng.dma_start(tabv[:, SBK * b:SBK * (b + 1), :], fused[:])

    # ---------------- gather + weighted sum + transpose out ---------------
    NGB = NT // GK
    for g in range(NGB):
        gO = gath.tile([128, GK * 2 * C], F16, tag="gO")
        gE = gath.tile([128, GK * 2 * C], F16, tag="gE")
        for gi, (gt, tab, idx) in enumerate(((gO, tabO, idxO),
                                             (gE, tabE, idxE))):
            nc.gpsimd.dma_gather(
                gt[:].rearrange("p (t e) -> p t e", e=2 * C),
                tab[:],
                idx[:, 8 * GK * g:8 * GK * (g + 1)],
                128 * GK, 128 * GK, 2 * C,
                single_packet=single_packet,
                queue_num=(2 * g + gi) % n_queues)
        if debug_taps is not None and g == 0:
            debug_taps["gO0"] = gO[:]
        gOv = gO[:].rearrange("p (t s c) -> p t s c", s=2, c=C)
        gEv = gE[:].rearrange("p (t s c) -> p t s c", s=2, c=C)
        for h2 in range(GK // K):
            tl = K * h2                  # local tile offset within batch
            t0 = GK * g + tl             # global tile offset
            acc = accp.tile([128, K * C], F16)
            tmp = accp.tile([128, K * C], F16)
            accv = acc[:].rearrange("p (k c) -> p k c", k=K)
            tmpv = tmp[:].rearrange("p (k c) -> p k c", k=K)
            terms = (
                (gEv, wEv, 0), (gEv, wEv, 1),
                (gOv, wOv, 0), (gOv, wOv, 1),
            )
            for ti, (gv, wv, sl) in enumerate(terms):
                dst = accv if ti == 0 else tmpv
                nc.vector.tensor_tensor(
                    dst,
                    gv[:, tl:tl + K, sl:sl + 1, :].squeeze(2),
                    wv[:, t0:t0 + K, sl:sl + 1].to_broadcast([128, K, C]),
                    op=MUL)
                if ti > 0:
                    nc.vector.tensor_tensor(accv, accv, tmpv, op=ADD)
            if debug_taps is not None and g == 0 and h2 == 0:
                debug_taps["acc0"] = acc[:]
            for half in range(2):
                hk = K // 2
                po = psumo.tile([C, hk * 128], F16)
                for t in range(hk):
                    th = hk * half + t
                    nc.tensor.transpose(po[:, 128 * t:128 * (t + 1)],
                                        acc[:, C * th:C * (th + 1)],
                                        ident16[:])
                ob = outp.tile([C, hk * 128], F32)
                nc.scalar.activation(ob[:], po[:],
                                     mybir.ActivationFunctionType.Copy)
                o0 = 128 * (t0 + hk * half)
                nc.sync.dma_start(out_ap[:, o0:o0 + 128 * hk], ob[:])
    return {"tabO": tabO, "tabE": tabE, "idxO": idxO, "idxE": idxE,
            "kpO": kpO, "kpE": kpE, "hf": hf}


def host_prep_gxy(grid_flat):
    """grid_flat [HW, 2] f32 -> [128, 2*NT] f32 (gx plane | gy plane)."""
    HW = grid_flat.shape[0]
    NT = HW // 128
    g = grid_flat.reshape(NT, 128, 2)
    return np.ascontiguousarray(
        np.concatenate([g[:, :, 0].T, g[:, :, 1].T], axis=1))




# ----------------------------------------------------------------------------
# self-contained kernel entry point
# ----------------------------------------------------------------------------
import concourse.bacc as bacc
from concourse.bass_utils import run_bass_kernel_spmd

N_CORES = 8
H = W = 256
C = 64
HW = H * W

_NC = None
LAST_RESULT = None


def _build_nc():
    global _NC
    if _NC is not None:
        return _NC
    nc = bacc.Bacc("TRN2", target_bir_lowering=False, debug=False)
    x = nc.dram_tensor("x", [C, HW], F32, kind="ExternalInput").ap()
    gxy = nc.dram_tensor("gxy", [128, 2 * (HW // 128)], F32,
                         kind="ExternalInput").ap()
    out = nc.dram_tensor("out", [C, HW], F32, kind="ExternalOutput").ap()
    with tile.TileContext(nc) as tc, ExitStack() as ctx:
        gs_body(ctx, tc, out, x, gxy, H=H, W=W)
    nc.compile()
    _NC = nc
    return nc


def kernel(x, grid, trace=False):
    global LAST_RESULT
    x = np.asarray(x, dtype=np.float32)
    grid = np.asarray(grid, dtype=np.float32)
    assert x.shape == (N_CORES, C, H, W) and grid.shape == (N_CORES, H, W, 2)
    nc = _build_nc()
    in_maps = []
    for n in range(N_CORES):
        in_maps.append({
            "x": np.ascontiguousarray(x[n].reshape(C, HW)),
            "gxy": host_prep_gxy(grid[n].reshape(HW, 2)),
        })
    res = run_bass_kernel_spmd(nc, in_maps, core_ids=list(range(N_CORES)),
                               trace=trace)
    LAST_RESULT = res
    out = np.stack([m["out"] for m in res.results])
    return out.reshape(N_CORES, C, H, W)


# revision 31
# speedup vs baseline: 2.1495x; 2.1495x over previous
"""Bilinear grid_sample (zeros padding, align_corners=False) Bass kernel for TRN2.

Per-core problem: x [64, H*W] f32 (NCHW flattened), gxy [128, 2*NT] f32
(host-transposed grid planes; cols 0:NT = gx, NT:2NT = gy, where plane[p, t]
= grid[t*128 + p]), out [64, H*W] f32.

Strategy (v2 — batched SWDGE dma_gather):
  Logical pair-table TBL[rho] = {upper: x_nhwc[rho-W-1], lower: x_nhwc[rho-1]}
  (zeros outside [0, HW)); per pixel r = (yc'+1)*W + xc' + 1 and the 4 bilinear
  taps are the upper/lower halves of TBL[r] and TBL[r+1].  To fit int16 gather
  indices, TBL is split into parity tables of 32768 rows each:
     O[k] = TBL[2k+1]   E[k] = TBL[2k+2]
  and the per-axis edge case (floor == size-1, where the +1 tap has zero
  weight) is remapped one step down so r <= 65535.  Each pixel gathers one O
  row (kO = floor(r/2)) and one E row (kE = max(0, floor((r-1)/2))); which
  table carries the {r} vs {r+1} slots depends on parity(r), folded into the
  per-pixel weights.  Tables are stored permuted (k' = (k%128)*NB + k//128) so
  the build writes 2 KB contiguous per partition.  Gathers run as large
  InstDMAGatherAnt batches (idx int16, replicated across the 8 Q7 16-partition
  stripes; idx j of a call sits at [16c + j%16, j//16] for all c).  Weighted
  sum on DVE, TensorE transpose back to [ch, px], ScalarE PSUM-evict cast
  fp16->f32, DMA out.
"""

from contextlib import ExitStack

import numpy as np

import concourse.bass as bass
import concourse.tile as tile
from concourse import mybir
from concourse.library_config import mlp
from concourse.masks import make_identity

F32 = mybir.dt.float32
F16 = mybir.dt.float16
I32 = mybir.dt.int32
I16 = mybir.dt.int16
MUL = mybir.AluOpType.mult
ADD = mybir.AluOpType.add
SUB = mybir.AluOpType.subtract
MAX = mybir.AluOpType.max
MIN = mybir.AluOpType.min
GE = mybir.AluOpType.is_ge
LE = mybir.AluOpType.is_le
GT = mybir.AluOpType.is_gt


def gs_body(ctx: ExitStack, tc: tile.TileContext, out_ap, x_ap, gxy_ap, *,
            H=256, W=256, GK=32, K=16, single_packet=False, n_queues=1,
            debug_taps=None):
    nc = tc.nc
    C = 64
    HW = H * W
    NT = HW // 128            # 128-px tiles (weight-plane columns)
    NROWS = HW // 2           # rows per parity table
    NB = NROWS // 128         # 128-row blocks per table
    SBK = 8                   # blocks per build superblock
    NSB = NB // SBK           # build superblocks
    SPX = HW // NSB           # x pixels per superblock

    nc.gpsimd.load_library(mlp)

    tabO = nc.dram_tensor("tabO", [NROWS, 2 * C], F16, kind="Internal").ap()
    tabE = nc.dram_tensor("tabE", [NROWS, 2 * C], F16, kind="Internal").ap()

    persist = ctx.enter_context(tc.tile_pool(name="persist", bufs=1))
    loadp = ctx.enter_context(tc.tile_pool(name="loadp", bufs=3))
    psumb = ctx.enter_context(tc.tile_pool(name="psumb", bufs=2, space="PSUM"))
    psumi = ctx.enter_context(tc.tile_pool(name="psumi", bufs=2, space="PSUM"))
    fusedp = ctx.enter_context(tc.tile_pool(name="fusedp", bufs=4))
    gath = ctx.enter_context(tc.tile_pool(name="gath", bufs=4))
    accp = ctx.enter_context(tc.tile_pool(name="accp", bufs=3))
    psumo = ctx.enter_context(tc.tile_pool(name="psumo", bufs=2, space="PSUM"))
    outp = ctx.enter_context(tc.tile_pool(name="outp", bufs=3))

    ident32 = persist.tile([128, 128], F32)
    make_identity(nc, ident32[:])
    ident16 = persist.tile([128, 128], F16)
    make_identity(nc, ident16[:])
    # S_q[p, m] = 1 iff p == 16q + m%16 (selectors for idx replication
    # matmuls; PE needs lhsT/rhs base partition 0 so contract over all 128)
    smat = persist.tile([128, 8 * 128], F32)
    for q in range(8):
        for rep in range(8):
            nc.vector.tensor_copy(
                smat[:, 128 * q + 16 * rep:128 * q + 16 * (rep + 1)],
                ident32[:, 16 * q:16 * (q + 1)])

    # ---------------- prologue: grid -> weights + gather indices ----------
    g_sb = persist.tile([128, 2 * NT], F32)
    nc.sync.dma_start(g_sb[:], gxy_ap[:])

    def ptile(dt, name):
        return persist.tile([128, NT], dt, name=name, tag=name)

    # Per-engine scratch planes: the x-axis chain runs on DVE while the
    # y-axis chain runs on GpSimd (idle before the gathers), so each engine
    # gets its own aliased temps.
    scratch = {}
    for pfx in ("vx", "gy"):
        scratch[pfx] = (ptile(I32, f"{pfx}_i"), ptile(F32, f"{pfx}_a"),
                        ptile(F32, f"{pfx}_b"), ptile(F32, f"{pfx}_c"))

    def ffloor(eng, pfx, dst_f32, v_ap):
        """dst = floor(v); v >= -1 smallish ints, exact in f32."""
        scr_i, ta, _, _ = scratch[pfx]
        eng.tensor_copy(scr_i[:], v_ap)
        eng.tensor_copy(dst_f32, scr_i[:])
        eng.tensor_tensor(ta[:], dst_f32, v_ap, op=GT)
        eng.tensor_tensor(dst_f32, dst_f32, ta[:], op=SUB)

    def axis_prep(eng, pfx, gsl, size, ax):
        """Return (c0, c1, clamped+edge-shifted floor in [-1, size-2])."""
        _, ta, tb, tc_ = scratch[pfx]
        v = ptile(F32, f"v_{ax}")       # dies at end of this call
        eng.tensor_scalar(v[:], gsl, size / 2.0, (size - 1) / 2.0, MUL, ADD)
        vf = ptile(F32, f"vf_{ax}")
        ffloor(eng, pfx, vf[:], v[:])
        t = ptile(F32, f"t_{ax}")
        eng.tensor_tensor(t[:], v[:], vf[:], op=SUB)
        eng.tensor_scalar(tb[:], vf[:], 0.0, None, GE)
        eng.tensor_scalar(tc_[:], vf[:], size - 1.0, None, LE)
        eng.tensor_tensor(tb[:], tb[:], tc_[:], op=MUL)     # valid0
        w0 = v                                              # reuse
        eng.tensor_scalar(w0[:], t[:], -1.0, 1.0, MUL, ADD)
        eng.tensor_tensor(w0[:], w0[:], tb[:], op=MUL)      # (1-t)*v0
        eng.tensor_scalar(tb[:], vf[:], -1.0, None, GE)
        eng.tensor_scalar(tc_[:], vf[:], size - 2.0, None, LE)
        eng.tensor_tensor(tb[:], tb[:], tc_[:], op=MUL)     # valid1
        eng.tensor_tensor(t[:], t[:], tb[:], op=MUL)        # t*v1
        eng.tensor_scalar(vf[:], vf[:], -1.0, size - 1.0, MAX, MIN)
        # edge shift: s = (vf == size-1) -> use the size-2 tap pair instead
        eng.tensor_scalar(tb[:], vf[:], size - 1.0, None, GE)   # s
        eng.tensor_scalar(tc_[:], tb[:], -1.0, 1.0, MUL, ADD)   # 1-s
        c0 = ptile(F32, f"c0_{ax}")
        eng.tensor_tensor(c0[:], w0[:], tc_[:], op=MUL)
        c1 = ptile(F32, f"c1_{ax}")
        eng.tensor_tensor(c1[:], t[:], tc_[:], op=MUL)
        eng.tensor_tensor(ta[:], w0[:], tb[:], op=MUL)      # w0*s
        eng.tensor_tensor(c1[:], c1[:], ta[:], op=ADD)
        eng.tensor_tensor(vf[:], vf[:], tb[:], op=SUB)
        return c0, c1, vf

    cx0, cx1, xc = axis_prep(nc.vector, "vx", g_sb[:, 0:NT], float(W), "x")
    cy0, cy1, yc = axis_prep(nc.vector, "gy", g_sb[:, NT:2 * NT], float(H), "y")

    # r = yc*W + xc + W + 1  in [0, 65535]
    gp = nc.vector
    gtb = scratch["gy"][2]
    rf = ptile(F32, "rf")
    gp.tensor_scalar(rf[:], yc[:], float(W), W + 1.0, MUL, ADD)
    gp.tensor_tensor(rf[:], rf[:], xc[:], op=ADD)
    # h = floor(r/2), par = r - 2h
    hf = ptile(F32, "hf")
    gp.tensor_scalar(gtb[:], rf[:], 0.5, None, MUL)
    ffloor(gp, "gy", hf[:], gtb[:])
    par = ptile(F32, "par")
    gp.tensor_scalar(par[:], hf[:], -2.0, None, MUL)
    gp.tensor_tensor(par[:], rf[:], par[:], op=ADD)
    # kO = h ; kE = max(0, h + par - 1)
    ke = rf                                                       # reuse
    gp.tensor_tensor(ke[:], hf[:], par[:], op=ADD)
    gp.tensor_scalar(ke[:], ke[:], -1.0, 0.0, ADD, MAX)

    def permute_rows(kf, name):
        """k' = (k % 128)*NB + k//128 (table stored partition-major)."""
        kp = ptile(F32, f"kp_{name}")
        gp.tensor_scalar(gtb[:], kf[:], 1.0 / 128.0, None, MUL)
        ffloor(gp, "gy", kp[:], gtb[:])                           # kp = k//128
        gp.tensor_scalar(gtb[:], kp[:], -128.0, None, MUL)
        gp.tensor_tensor(gtb[:], kf[:], gtb[:], op=ADD)           # k%128
        gp.tensor_scalar(gtb[:], gtb[:], float(NB), None, MUL)
        gp.tensor_tensor(kp[:], gtb[:], kp[:], op=ADD)
        return kp

    kpO = permute_rows(hf, "O")
    kpE = permute_rows(ke, "E")

    # slot weights + parity blend (fp16, interleaved [p, (t s)] with s=up,lo)
    s0 = ptile(F32, "s0")
    s1 = ptile(F32, "s1")
    s2 = ptile(F32, "s2")
    s3 = ptile(F32, "s3")
    nc.vector.tensor_tensor(s0[:], cy0[:], cx0[:], op=MUL)
    nc.vector.tensor_tensor(s1[:], cy1[:], cx0[:], op=MUL)
    nc.vector.tensor_tensor(s2[:], cy0[:], cx1[:], op=MUL)
    nc.vector.tensor_tensor(s3[:], cy1[:], cx1[:], op=MUL)
    omp = ptile(F32, "omp")
    nc.vector.tensor_scalar(omp[:], par[:], -1.0, 1.0, MUL, ADD)
    wE = persist.tile([128, NT * 2], F16)
    wO = persist.tile([128, NT * 2], F16)
    wEv = wE[:].rearrange("p (t s) -> p t s", s=2)
    wOv = wO[:].rearrange("p (t s) -> p t s", s=2)
    # E carries {S0,S1} when r even, {S2,S3} when r odd; O the reverse.
    vta, vtb = scratch["vx"][1], scratch["vx"][2]
    for dstv, a_even, a_odd in (
        (wEv[:, :, 0:1], s0, s2),
        (wEv[:, :, 1:2], s1, s3),
        (wOv[:, :, 0:1], s2, s0),
        (wOv[:, :, 1:2], s3, s1),
    ):
        nc.vector.tensor_tensor(vta[:], a_even[:], omp[:], op=MUL)
        nc.vector.tensor_tensor(vtb[:], a_odd[:], par[:], op=MUL)
        nc.vector.tensor_tensor(vta[:], vta[:], vtb[:], op=ADD)
        nc.vector.tensor_copy(dstv.squeeze(2), vta[:])

    if debug_taps is not None:
        debug_taps["wE"] = wE[:]
        debug_taps["wO"] = wO[:]

    # ---------------- idx planes: int16, stripe-replicated ----------------
    # idx for call-slot j lives at [16c + j%16, j//16] for every stripe c.
    # j = t*128 + p  ->  partition (j%16)=p%16 rep 8x, column 8t + p//16.
    idxO = persist.tile([128, 8 * NT], I16)
    idxE = persist.tile([128, 8 * NT], I16)
    for kp, idx in ((kpO, idxO), (kpE, idxE)):
        idxv = idx[:].rearrange("p (t q) -> p t q", q=8)
        for q in range(8):
            pm = psumi.tile([128, NT], F32, tag="pmidx")
            nc.tensor.matmul(out=pm[:], lhsT=smat[:, 128 * q:128 * (q + 1)],
                             rhs=kp[:], start=True, stop=True)
            nc.vector.tensor_copy(idxv[:, :, q:q + 1].squeeze(2), pm[:])

    # ---------------- build the parity tables -----------------------------
    tabOv = tabO.rearrange("(p B) e -> p B e", p=128)
    tabEv = tabE.rearrange("(p B) e -> p B e", p=128)
    prev_ps = [None, None]
    for b in range(NSB):
        xs = loadp.tile([C, SPX], F32)
        nc.sync.dma_start(xs[:], x_ap[:, SPX * b:SPX * (b + 1)])
        xsv = xs[:].rearrange("c (px two) -> c px two", two=2)
        for parity, tabv in ((0, tabOv), (1, tabEv)):
            ps = psumb.tile([128, SBK * C], F32, tag=f"ps{parity}")
            for j in range(SBK):
                nc.tensor.transpose(
                    ps[:, C * j:C * (j + 1)],
                    xsv[:, 128 * j:128 * (j + 1), parity:parity + 1].squeeze(2),
                    ident32[0:C, 0:C])
            psv = ps[:].rearrange("p (j c) -> p j c", c=C)
            fused = fusedp.tile([128, SBK, 2 * C], F16, tag=f"fu{parity}")
            # lowers / uppers split across ACT and DVE, alternating by parity
            if parity == 0:
                nc.scalar.activation(fused[:, :, C:2 * C], psv,
                                     mybir.ActivationFunctionType.Copy)
            else:
                nc.vector.tensor_copy(fused[:, :, C:2 * C], psv)
            # uppers j>=1 from this superblock, j=0 from the previous one
            if parity == 0:
                nc.vector.tensor_copy(fused[:, 1:SBK, 0:C], psv[:, 0:SBK - 1, :])
            else:
                nc.scalar.activation(fused[:, 1:SBK, 0:C], psv[:, 0:SBK - 1, :],
                                     mybir.ActivationFunctionType.Copy)
            if b == 0:
                nc.gpsimd.memset(fused[:, 0:1, 0:C], 0.0)
            elif parity == 0:
                nc.vector.tensor_copy(fused[:, 0:1, 0:C],
                                      prev_ps[parity][:, SBK - 1:SBK, :])
            else:
                nc.scalar.activation(fused[:, 0:1, 0:C],
                                     prev_ps[parity][:, SBK - 1:SBK, :],
                                     mybir.ActivationFunctionType.Copy)
            prev_ps[parity] = psv
            eng = nc.sync if parity == 0 else nc.scalar
            eng.dma_start(tabv[:, SBK * b:SBK * (b + 1), :], fused[:])

    # ---------------- gather + weighted sum + transpose out ---------------
    NGB = NT // GK
    for g in range(NGB):
        gO = gath.tile([128, GK * 2 * C], F16, tag="gO")
        gE = gath.tile([128, GK * 2 * C], F16, tag="gE")
        for gi, (gt, tab, idx) in enumerate(((gO, tabO, idxO),
                                             (gE, tabE, idxE))):
            nc.gpsimd.dma_gather(
                gt[:].rearrange("p (t e) -> p t e", e=2 * C),
                tab[:],
                idx[:, 8 * GK * g:8 * GK * (g + 1)],
                128 * GK, 128 * GK, 2 * C,
                single_packet=single_packet,
                queue_num=(2 * g + gi) % n_queues)
        if debug_taps is not None and g == 0:
            debug_taps["gO0"] = gO[:]
        gOv = gO[:].rearrange("p (t s c) -> p t s c", s=2, c=C)
        gEv = gE[:].rearrange("p (t s c) -> p t s c", s=2, c=C)
        for h2 in range(GK // K):
            tl = K * h2                  # local tile offset within batch
            t0 = GK * g + tl             # global tile offset
            acc = accp.tile([128, K * C], F16)
            tmp = accp.tile([128, K * C], F16)
            accv = acc[:].rearrange("p (k c) -> p k c", k=K)
            tmpv = tmp[:].rearrange("p (k c) -> p k c", k=K)
            terms = (
                (gEv, wEv, 0), (gEv, wEv, 1),
                (gOv, wOv, 0), (gOv, wOv, 1),
            )
            for ti, (gv, wv, sl) in enumerate(terms):
                dst = accv if ti == 0 else tmpv
                nc.vector.tensor_tensor(
                    dst,
                    gv[:, tl:tl + K, sl:sl + 1, :].squeeze(2),
                    wv[:, t0:t0 + K, sl:sl + 1].to_broadcast([128, K, C]),
                    op=MUL)
                if ti > 0:
                    nc.vector.tensor_tensor(accv, accv, tmpv, op=ADD)
            if debug_taps is not None and g == 0 and h2 == 0:
                debug_taps["acc0"] = acc[:]
            for half in range(2):
                hk = K // 2
                po = psumo.tile([C, hk * 128], F16)
                for t in range(hk):
                    th = hk * half + t
                    nc.tensor.transpose(po[:, 128 * t:128 * (t + 1)],
                                        acc[:, C * th:C * (th + 1)],
                                        ident16[:])
                ob = outp.tile([C, hk * 128], F32)
                nc.scalar.activation(ob[:], po[:],
                                     mybir.ActivationFunctionType.Copy)
                o0 = 128 * (t0 + hk * half)
                nc.sync.dma_start(out_ap[:, o0:o0 + 128 * hk], ob[:])
    return {"tabO": tabO, "tabE": tabE, "idxO": idxO, "idxE": idxE,
            "kpO": kpO, "kpE": kpE, "hf": hf}


def host_prep_gxy(grid_flat):
    """grid_flat [HW, 2] f32 -> [128, 2*NT] f32 (gx plane | gy plane)."""
    HW = grid_flat.shape[0]
    NT = HW // 128
    g = grid_flat.reshape(NT, 128, 2)
    return np.ascontiguousarray(
        np.concatenate([g[:, :, 0].T, g[:, :, 1].T], axis=1))




# ----------------------------------------------------------------------------
# self-contained kernel entry point
# ----------------------------------------------------------------------------
import concourse.bacc as bacc
from concourse.bass_utils import run_bass_kernel_spmd

N_CORES = 8
H = W = 256
C = 64
HW = H * W

_NC = None
LAST_RESULT = None


def _build_nc():
    global _NC
    if _NC is not None:
        return _NC
    nc = bacc.Bacc("TRN2", target_bir_lowering=False, debug=False,
                   num_swdge_queues=4)
    x = nc.dram_tensor("x", [C, HW], F32, kind="ExternalInput").ap()
    gxy = nc.dram_tensor("gxy", [128, 2 * (HW // 128)], F32,
                         kind="ExternalInput").ap()
    out = nc.dram_tensor("out", [C, HW], F32, kind="ExternalOutput").ap()
    with tile.TileContext(nc) as tc, ExitStack() as ctx:
        gs_body(ctx, tc, out, x, gxy, H=H, W=W, GK=8, K=8, n_queues=4)
    nc.compile()
    _NC = nc
    return nc


def kernel(x, grid, trace=False):
    global LAST_RESULT
    x = np.asarray(x, dtype=np.float32)
    grid = np.asarray(grid, dtype=np.float32)
    assert x.shape == (N_CORES, C, H, W) and grid.shape == (N_CORES, H, W, 2)
    nc = _build_nc()
    in_maps = []
    for n in range(N_CORES):
        in_maps.append({
            "x": np.ascontiguousarray(x[n].reshape(C, HW)),
            "gxy": host_prep_gxy(grid[n].reshape(HW, 2)),
        })
    res = run_bass_kernel_spmd(nc, in_maps, core_ids=list(range(N_CORES)),
                               trace=trace)
    LAST_RESULT = res
    out = np.stack([m["out"] for m in res.results])
    return out.reshape(N_CORES, C, H, W)
